# revision 19
# baseline (speedup 1.0000x reference)
"""GQA forward (B=2,T=2048,E=2048,H=32,HKV=8,D=64, RoPE, causal) on 8 trn2 cores.

Sharding: tensor-parallel over kv-heads. Core c owns kv-head c and q-heads
4c..4c+3 (columns 256c:256c+256 of Wq, 64c:64c+64 of Wk/Wv, rows
256c:256c+256 of Wo). Each core computes its heads' attention for both
batches plus the partial o-projection y_c @ Wo_c; the host sums the 8
partials.

v2 layout (all feature-major, x pre-transposed on host):
  x^T  [e=128, ec=16, t]    plain DMA (host supplies x^T -> no DMA transpose)
  Q^T  per head-pair hp: [64, t, g] bf16, heads 2hp/2hp+1 column-interleaved
       so one scores matmul covers both heads (fewer, wider PE ops).
  K^T  [64, t] bf16; V^T staged [64, t] then PE-transposed to row-layout
       vS blocks [128 keys, 64+1] (col 64 = ones -> softmax denom).
  scores sp [128 keys, tq*2] = kT_blk.T @ qTi per key block, split in two
       512-col matmuls (PSUM bank limit); causal -1e9 mask added via
       identity x maskbias matmul into the same PSUM group on diag blocks.
  P = exp(0.125*sp) bf16 (scalar engine, one wide op per block),
  y^T [65, tq*2] += vS.T @ P (two 512-col matmuls, deferred 2 blocks so
       the PE never waits on exp).
  normalize: reciprocal_approx_fast (DVE) -> partition_broadcast (gpsimd)
       -> two strided DVE muls de-interleave into yT [128, t].
  out chunk [t=128, e] = yT_chunk.T @ Wo, accumulated over the 2 head-pairs,
       stores interleaved with attention per q-chunk.
No softmax max-subtraction: scores*0.125 are O(5), exp stays in fp32 range.
"""
import os

import numpy as np
import ml_dtypes

import concourse.mybir as mybir
import concourse.tile as tile
from concourse import bacc
from concourse.bass_utils import run_bass_kernel_spmd

F32 = mybir.dt.float32
BF16 = mybir.dt.bfloat16
AF = mybir.ActivationFunctionType
BF16NP = ml_dtypes.bfloat16

B, T, E = 2, 2048, 2048
H, HKV, D = 32, 8, 64
G = H // HKV          # q heads per kv head (= per core)
NCORES = 8
QH = G * D            # 256 q cols per core
ECH = E // 128        # 16 contraction chunks
TH = 1024             # proj half-batch width
TQC = 512             # attention q-chunk width
NTQ = T // TQC        # 4
ROPE_BASE = 10000.0
NEG = -1.0e9

_compiled = None
LAST_RESULT = None


def _build():
    nc = bacc.Bacc(None, target_bir_lowering=False, debug=False)

    x_d = nc.declare_dram_parameter("x", [E, B * T], BF16, isOutput=False)   # x^T
    wq_d = nc.declare_dram_parameter("wq", [E, QH], BF16, isOutput=False)
    wkv_d = nc.declare_dram_parameter("wkv", [E, 2 * D], BF16, isOutput=False)
    wo_d = nc.declare_dram_parameter("wo", [QH, E], BF16, isOutput=False)
    cs_d = nc.declare_dram_parameter("cs", [128, T], F32, isOutput=False)
    mb_d = nc.declare_dram_parameter("maskb", [128, 128, 2], BF16, isOutput=False)
    id_d = nc.declare_dram_parameter("ident", [128, 128], BF16, isOutput=False)
    out_d = nc.declare_dram_parameter("out", [B * T, E], BF16, isOutput=True)

    with tile.TileContext(nc) as tc:
        with (
            tc.tile_pool(name="const", bufs=1) as cp,
            tc.tile_pool(name="acts", bufs=1) as ac,
            tc.tile_pool(name="xT", bufs=2) as xp,
            tc.tile_pool(name="work", bufs=2) as wp,
            tc.tile_pool(name="ps", bufs=2, space="PSUM") as ps,
        ):
            wq_sb = cp.tile([128, ECH, QH], BF16)
            nc.sync.dma_start(wq_sb[:], wq_d[:, :].rearrange("(c p) n -> p c n", p=128))
            wkv_sb = cp.tile([128, ECH, 2 * D], BF16)
            nc.sync.dma_start(wkv_sb[:], wkv_d[:, :].rearrange("(c p) n -> p c n", p=128))
            wo_sb = cp.tile([128, 2, E], BF16)
            nc.sync.dma_start(wo_sb[:], wo_d[:, :].rearrange("(c p) n -> p c n", p=128))
            cs_sb = cp.tile([128, T], F32)          # rows 0:64 cos, 64:128 [-sin;sin]
            nc.sync.dma_start(cs_sb[:], cs_d[:, :])
            mb_sb = cp.tile([128, 128, 2], BF16)    # causal -1e9 bias, head-interleaved
            nc.sync.dma_start(mb_sb[:], mb_d[:, :, :])
            id_sb = cp.tile([128, 128], BF16)
            nc.sync.dma_start(id_sb[:], id_d[:, :])

            qTi, kT, vS, yT, vTt = {}, {}, {}, {}, {}
            for b in range(B):
                kT[b] = ac.tile([64, T], BF16, name=f"kT{b}", tag=f"kT{b}")
                vTt[b] = ac.tile([64, T], BF16, name=f"vTt{b}", tag=f"vTt{b}")
                for i in range(T // 128):
                    vS[b, i] = ac.tile([128, D + 1], BF16, name=f"vS{b}_{i}",
                                       tag=f"vS{b}_{i}")
                    nc.vector.memset(vS[b, i][:, D:D + 1], 1.0)
                for hp in range(2):
                    qTi[b, hp] = ac.tile([64, T, 2], BF16, name=f"qTi{b}{hp}",
                                         tag=f"qTi{b}{hp}")
                    yT[b, hp] = ac.tile([128, T], BF16, name=f"yT{b}{hp}",
                                        tag=f"yT{b}{hp}")

            def fl(ap):
                return ap.rearrange("p a b -> p (a b)")

            def rope(dst, psrc, r0, c0, w):
                # dst[64, w] <- rope(psum[r0:r0+64, 0:w]); cs cols c0:c0+w
                t1 = wp.tile([64, TH], F32, tag="rt", bufs=4)
                t2 = wp.tile([64, TH], F32, tag="rt", bufs=4)
                nc.vector.tensor_mul(t1[:, 0:w], psrc[r0:r0 + 64, 0:w],
                                     cs_sb[0:64, c0:c0 + w])
                nc.vector.tensor_mul(t2[0:32, 0:w], psrc[r0 + 32:r0 + 64, 0:w],
                                     cs_sb[64:96, c0:c0 + w])
                nc.vector.tensor_mul(t2[32:64, 0:w], psrc[r0:r0 + 32, 0:w],
                                     cs_sb[96:128, c0:c0 + w])
                nc.vector.tensor_add(dst, t1[:, 0:w], t2[:, 0:w])

            pending = [None]
            pending_vt = [None]
            pending_op = []

            def flush():
                if pending[0] is not None:
                    pending[0]()
                    pending[0] = None

            def flush_vt():
                if pending_vt[0] is not None:
                    pending_vt[0]()
                    pending_vt[0] = None

            def flush_op(n=1):
                for _ in range(min(n, len(pending_op))):
                    pending_op.pop(0)()

            for b in range(B):
                # ---- projections: Q (rope, pair-interleaved), K (rope), V ----
                for h in range(2):
                    n0 = b * T + h * TH
                    t0 = h * TH
                    xT = xp.tile([128, ECH, TH], BF16, tag="xT")
                    xr = x_d[:, :].rearrange("(c p) n -> p c n", p=128)
                    for ec in range(ECH):
                        nc.sync.dma_start(xT[:, ec, :], xr[:, ec, n0:n0 + TH])
                    for hp in range(2):
                        qp = ps.tile([128, TH], F32, tag="w", bufs=3)
                        for bb in range(2):
                            for ec in range(ECH):
                                nc.tensor.matmul(
                                    qp[:, 512 * bb:512 * bb + 512],
                                    wq_sb[:, ec, 128 * hp:128 * hp + 128],
                                    xT[:, ec, 512 * bb:512 * bb + 512],
                                    start=(ec == 0), stop=(ec == ECH - 1))
                        if hp == 0:
                            flush_op(99)
                        else:
                            flush_vt()
                        rope(qTi[b, hp][0:64, t0:t0 + TH, 0], qp, 0, t0, TH)
                        rope(qTi[b, hp][0:64, t0:t0 + TH, 1], qp, 64, t0, TH)
                    kv = ps.tile([128, TH], F32, tag="w", bufs=3)
                    for bb in range(2):
                        for ec in range(ECH):
                            nc.tensor.matmul(kv[:, 512 * bb:512 * bb + 512],
                                             wkv_sb[:, ec, :],
                                             xT[:, ec, 512 * bb:512 * bb + 512],
                                             start=(ec == 0), stop=(ec == ECH - 1))
                    rope(kT[b][:, t0:t0 + TH], kv, 0, t0, TH)
                    nc.scalar.activation(vTt[b][:, t0:t0 + TH], kv[64:128, :], AF.Copy)

                    def vt(b=b, h=h, t0=t0):
                        for blk in range(TH // 128):
                            gi = h * (TH // 128) + blk
                            vtp = ps.tile([128, 64], BF16, tag="w", bufs=3)
                            nc.tensor.transpose(
                                vtp[:, :],
                                vTt[b][:, t0 + blk * 128:t0 + (blk + 1) * 128],
                                id_sb[0:64, 0:64])
                            nc.scalar.activation(vS[b, gi][:, 0:D], vtp[:, :],
                                                 AF.Copy)
                    pending_vt[0] = vt

                # ---- attention + o-projection, chunk by chunk ----
                for j in range(NTQ):
                    tc0 = j * TQC
                    for hp in range(2):
                        yp = ps.tile([65, TQC, 2], F32, tag="y", bufs=1)
                        nblk = 4 * j + 4
                        stop0 = min(4 * j + 1, nblk - 1)   # last block touching H0
                        ydefer = []

                        def emit_y(yp=yp, nblk=nblk, stop0=stop0):
                            sb, t0q, pt = ydefer.pop(0)
                            if t0q < 256:
                                nc.tensor.matmul(
                                    fl(yp[:, t0q:256, :]), vS[b, sb][:, :],
                                    fl(pt[:, t0q:256, :]),
                                    start=(sb == 0), stop=(sb == stop0),
                                    skip_group_check=True)
                            nc.tensor.matmul(
                                fl(yp[:, max(256, t0q):TQC, :]), vS[b, sb][:, :],
                                fl(pt[:, max(256, t0q):TQC, :]),
                                start=(sb == 0), stop=(sb == nblk - 1),
                                skip_group_check=True)

                        for sb in range(nblk):
                            k = sb - 4 * j
                            j0 = 128 * k if k > 0 else 0   # valid-from (t units)
                            if sb == 0:
                                # previous group's normalize chain first (the
                                # deferred o-proj units below read its yT)
                                flush()
                            flush_op(2 if sb == 1 else 1)
                            if sb == 2 and hp == 0 and j == 1:
                                flush_vt()
                            sp = ps.tile([128, TQC, 2], F32, tag="w", bufs=3)
                            # scores in two 256-t (512-col) PSUM-bank groups
                            for bb in range(2):
                                lo = max(j0, 256 * bb)
                                hi = 256 * (bb + 1)
                                if lo >= hi:
                                    continue
                                corner = (k >= 0) and (256 * bb <= j0 < hi)
                                nc.tensor.matmul(
                                    fl(sp[:, lo:hi, :]),
                                    kT[b][:, 128 * sb:128 * sb + 128],
                                    fl(qTi[b, hp][0:64, tc0 + lo:tc0 + hi, :]),
                                    start=True, stop=(not corner),
                                    skip_group_check=True)
                                if corner:
                                    nc.tensor.matmul(
                                        fl(sp[:, j0:j0 + 128, :]), id_sb[:, :],
                                        fl(mb_sb[:, :, :]),
                                        start=False, stop=True,
                                        skip_group_check=True)
                            pt = wp.tile([128, TQC, 2], BF16, tag="pt", bufs=5)
                            nc.scalar.activation(fl(pt[:, j0:TQC, :]),
                                                 fl(sp[:, j0:TQC, :]),
                                                 AF.Exp, scale=0.125)
                            ydefer.append((sb, j0, pt))
                            if sb >= 3:
                                emit_y()
                        while ydefer:
                            emit_y()

                        def tail(yp=yp, hp=hp, tc0=tc0):
                            dn = wp.tile([1, TQC, 2], F32, tag="dn", bufs=1)
                            nc.scalar.activation(fl(dn[:, :, :]),
                                                 fl(yp[64:65, :, :]), AF.Copy)
                            rc = wp.tile([1, TQC, 2], F32, tag="rc", bufs=1)
                            nc.vector.reciprocal_approx_fast(rc[:, :, :],
                                                             dn[:, :, :])
                            yf = wp.tile([64, TQC, 2], F32, tag="rt", bufs=4)
                            nc.vector.tensor_copy(yf[:, :, :], yp[0:64, :, :])
                            bcs = wp.tile([64, TQC, 2], F32, tag="bc", bufs=1)
                            nc.gpsimd.partition_broadcast(fl(bcs[:, :, :]),
                                                          fl(rc[:, :, :]))
                            for g in range(2):
                                nc.vector.tensor_mul(
                                    yT[b, hp][64 * g:64 * g + 64, tc0:tc0 + TQC],
                                    yf[:, :, g], bcs[:, :, g])
                        pending[0] = tail
                    for tb in range(4):
                        r0 = tc0 + tb * 128
                        for eh in range(2):
                            def op_unit(b=b, r0=r0, eh=eh):
                                op = ps.tile([128, TH], F32, tag="w", bufs=3)
                                for bb in range(2):
                                    e0 = TH * eh + 512 * bb
                                    for hc in range(2):
                                        nc.tensor.matmul(
                                            op[:, 512 * bb:512 * bb + 512],
                                            yT[b, hc][:, r0:r0 + 128],
                                            wo_sb[:, hc, e0:e0 + 512],
                                            start=(hc == 0), stop=(hc == 1))
                                ot = wp.tile([128, TH], BF16, tag="ot", bufs=2)
                                if eh == 0:
                                    nc.scalar.activation(ot[:], op[:], AF.Copy)
                                else:
                                    nc.vector.tensor_copy(ot[:], op[:])
                                nc.sync.dma_start(
                                    out_d[b * T + r0:b * T + r0 + 128,
                                          TH * eh:TH * eh + TH],
                                    ot[:])
                            pending_op.append(op_unit)
                flush()
            flush()
            flush_op(99)

    nc.compile()
    return nc


def _host_consts():
    inv = ROPE_BASE ** (-np.arange(32, dtype=np.float64) / 32.0)
    ang = np.outer(inv, np.arange(T, dtype=np.float64))          # [32, T]
    cos64 = np.tile(np.cos(ang), (2, 1))
    sin32 = np.sin(ang)
    sinS = np.concatenate([-sin32, sin32], axis=0)
    cs = np.concatenate([cos64, sinS], axis=0).astype(np.float32)   # [128, T]
    mb = np.zeros((128, 128, 2), np.float32)
    p = np.arange(128)
    invalid = p[:, None] > p[None, :]                            # key p > query t
    mb[:, :, 0] = np.where(invalid, NEG, 0.0)
    mb[:, :, 1] = mb[:, :, 0]
    ident = np.eye(128, dtype=np.float32)
    return cs, mb.astype(BF16NP), ident.astype(BF16NP)


def kernel(x, Wq, Wk, Wv, Wo):
    global _compiled, LAST_RESULT
    if _compiled is None:
        _compiled = _build()
    nc = _compiled

    xT = np.ascontiguousarray(
        np.asarray(x, np.float32).reshape(B * T, E).T).astype(BF16NP)
    cs, mb, ident = _host_consts()
    in_maps = []
    for c in range(NCORES):
        wkv = np.concatenate(
            [Wk[:, D * c:D * (c + 1)], Wv[:, D * c:D * (c + 1)]], axis=1)
        in_maps.append({
            "x": xT,
            "wq": np.ascontiguousarray(Wq[:, QH * c:QH * (c + 1)]).astype(BF16NP),
            "wkv": np.ascontiguousarray(wkv).astype(BF16NP),
            "wo": np.ascontiguousarray(Wo[QH * c:QH * (c + 1), :]).astype(BF16NP),
            "cs": cs,
            "maskb": mb,
            "ident": ident,
        })
    trace = os.environ.get("GQA_TRACE", "0") == "1"
    res = run_bass_kernel_spmd(nc, in_maps, core_ids=list(range(NCORES)), trace=trace)
    LAST_RESULT = res
    acc = np.zeros((B * T, E), np.float32)
    for r in res.results:
        acc += np.asarray(r["out"]).astype(np.float32)
    return acc.reshape(B, T, E)
